# revision 26
# baseline (speedup 1.0000x reference)
"""Gaussian-splat differentiable renderer on 8 TRN2 NeuronCores.

The reference renders N=4096 isotropic 2D gaussians into a 128x128 image
but returns only the first 1024 pixels (y in [0,8), x in [0,128)) per
batch.  The gaussians are isotropic and pixels live on a grid, so the
weight separates: w[n,(x,y)] = g(n,x) * f(n,y), g = exp(-((x-u)*sd)^2),
f = exp(-((y-v)*sd)^2), sd = sqrt(0.5)/scale.

Sharding: 8 cores = batch (2) x x-blocks of 32 columns (4).  Each core
holds all N gaussians (partition p, chunk k; n = p*32+k) and owns its 32
x-columns end to end -- no collectives.  Host prep folds all O(N)
per-gaussian linear algebra (camera transform, projection, footprint
scaling) into the input layout; the device does the O(N x pixels)
render: subtract grids, exponentiate, weight-by-color, and the 4M-MAC
num/den accumulation per core.

Device-side structure (v8):
 - the g-side exp(-d^2) (the N x W bulk) via the ACT engine's
   Derivative_Erf table (d/dx erf = 2/sqrt(pi) e^{-x^2}); the 4/pi
   product factor is folded into the opacity-premultiplied colors
   host-side, and the small f-side factor (N x 8) rides in with the
   host prep.  DVE runs only stock fp16 SUBTRACTs (2 elem/cycle mode),
   with the gaussian-chunk axis k LAST so broadcasts land on middle
   dims.
 - two DMAs, one per HWDGE ring: sync carries ui+XI (everything the
   g path and hence the ACT chain needs); scalar carries EFH+OC
   (needed only by T3, which has slack behind the ACT table loads).
 - the 32 PSUM-accumulated fp16 matmuls rotate the stationary operand
   across the four 32-column PE groups (tile_position) so each chunk's
   LDWEIGHTS overlaps the previous chunk's MATMUL in a different
   sub-array; the four partial accumulators land in four partition
   blocks of one PSUM tile.
 - raw 4x(num|den) goes straight out (ACT copies PSUM->SBUF); the
   group-merge and final division happen host-side during unsharding.
 - ~115 dummy matmuls warm the PE clock (HAM) before the real stream.
"""

import numpy as np

N_GAUSS = 4096
P = 128          # partitions
KC = 32          # gaussian chunks along the free axis (n = p*KC + k)
KH = 16          # half of KC (ACT/matmul pipeline granularity)
NX = 32          # x columns per core
NY = 8           # y rows in the output
N_CORES = 8
SQ2I = 0.7071067811865476
PI4 = 0.7853981633974483   # pi/4, cancels the (2/sqrt(pi))^2 of D_Erf^2
N_WARM_MM = 115

_BUILT = {}


def _quat2mat(q):
    q = q.astype(np.float32)
    q = q / np.float32(np.sqrt(np.float32((q * q).sum())))
    w, x, y, z = [np.float32(v) for v in q]
    return np.array(
        [
            [1 - 2 * (y * y + z * z), 2 * (x * y - z * w), 2 * (x * z + y * w)],
            [2 * (x * y + z * w), 1 - 2 * (x * x + z * z), 2 * (y * z - x * w)],
            [2 * (x * z - y * w), 2 * (y * z + x * w), 1 - 2 * (x * x + y * y)],
        ],
        np.float32,
    )


def _build():
    key = "nc"
    if key in _BUILT:
        return _BUILT[key]

    import concourse.mybir as mybir
    import concourse.tile as tile
    from concourse import bacc
    from concourse.tile_rust import add_dep_helper

    f32 = mybir.dt.float32
    f16 = mybir.dt.float16
    DERF = mybir.ActivationFunctionType.Derivative_Erf
    COPY = mybir.ActivationFunctionType.Copy

    nc = bacc.Bacc("TRN2", target_bir_lowering=False, debug=False,
                   enable_asserts=False, num_devices=N_CORES)

    # rows 0-7: EFH[y,k] = (2/sqrt(pi))*exp(-(YI-vi)^2);
    # rows 8-11: OC' = (pi/4)*opa*(r,g,b,1)
    yu_d = nc.dram_tensor("yu", [P, 12, KC], f16, kind="ExternalInput")
    # row 0: ui (projected x center in sd units);
    # rows 1-32: XI[p, x, k] = (x + 32*xb - cx)*c*iss[p,k]   (k LAST)
    xi_d = nc.dram_tensor("xi", [P, 1 + NX, KC], f16, kind="ExternalInput")
    # rows 32g+x: group-g partial; cols 0:32 bank-A and 32:64 bank-B
    # partials, each (d,y) d-major, 24 num + 8 den
    out_d = nc.dram_tensor("out", [P, 64], f32, kind="ExternalOutput")

    with tile.TileContext(nc) as tc:
        with (
            tc.tile_pool(name="sb", bufs=1) as pool,
            tc.tile_pool(name="ps", bufs=1, space="PSUM") as psum,
        ):
            # PE warm-up: independent matmuls keep the PE activity window
            # hot so the real stream runs at 2.4 GHz.
            DW = pool.tile([P, NX], f16)
            PSDB = psum.tile([NX, 512], f32)
            PSD = PSDB[:, 0:NX]
            nc.gpsimd.memset(DW[:], 0.25)
            for _ in range(N_WARM_MM):
                nc.tensor.matmul(PSD, DW[:], DW[:], start=True, stop=True)

            YU = pool.tile([P, 12, KC], f16)
            XU = pool.tile([P, 1 + NX, KC], f16)
            # one DMA per HWDGE ring; the whole g path (which gates the
            # ACT chain) depends only on the sync-ring tensor, while the
            # scalar-ring tensor is needed only by T3 (which has slack)
            nc.sync.dma_start(XU[:], xi_d[:])
            nc.scalar.dma_start(YU[:], yu_d[:])

            EFH = YU[:, 0:NY, :]
            OC = YU[:, NY : NY + 4, :]

            EGA = pool.tile([P, NX, KC], f16)
            EGH = pool.tile([P, NX, KC], f16)
            T3 = pool.tile([P, 4, NY, KC], f16)
            # two full-bank accumulators: with 4 column groups x 2 banks
            # the same (group, bank) pair repeats only every 8 chunks,
            # and the two banks drain in parallel (ACT + DVE)
            PSA = psum.tile([P, 512], f32)
            PSB = psum.tile([P, 512], f32)

            # g path halves: d = XI - UI (k-last keeps broadcasts mid-dim)
            g_subs = []
            for s in range(2):
                ks = slice(s * KH, (s + 1) * KH)
                g_subs.append(nc.vector.tensor_sub(
                    EGA[:, :, ks], XU[:, 1 : 1 + NX, ks],
                    XU[:, 0, None, ks].broadcast_to([P, NX, KH])))

            for s in range(2):
                ks = slice(s * KH, (s + 1) * KH)
                nc.scalar.activation(EGH[:, :, ks], EGA[:, :, ks], DERF)

            # T3[p, d, y, k] = EFH[p, y, k] * OC[p, d, k], on DVE (2x);
            # scheduling-only dep keeps the subs ahead of T3 on the DVE
            # queue even if the model mispredicts the two DMA latencies
            for s in range(2):
                ks = slice(s * KH, (s + 1) * KH)
                t3_op = nc.vector.tensor_mul(
                    T3[:, :, :, ks],
                    EFH[:, None, :, ks].broadcast_to([P, 4, NY, KH]),
                    OC[:, :, None, ks].broadcast_to([P, 4, NY, KH]),
                )
                add_dep_helper(t3_op.ins, g_subs[1].ins, sync=False,
                               reason="subs before T3 on DVE")

            # rotate chunks across the 4 PE column groups: LDW(k+1)
            # overlaps MM(k) in a different 32x32 sub-array column strip
            for k in range(KC):
                g = k & 3
                PSb = PSA if (k & 4) == 0 else PSB
                nc.tensor.matmul(
                    PSb[32 * g : 32 * (g + 1), 0:32], EGH[:, :, k],
                    T3[:, :, :, k].rearrange("x a b -> x (a b)"),
                    start=(k < 8), stop=(k >= KC - 8),
                    tile_position=(0, 32 * g),
                )

            # raw group partials to DRAM (merge + division on the host);
            # the two banks drain concurrently on ACT and DVE
            S = pool.tile([P, 64], f32)
            nc.scalar.activation(S[:, 0:32], PSA[:, 0:32], COPY)
            nc.vector.tensor_copy(S[:, 32:64], PSB[:, 0:32])
            nc.sync.dma_start(out_d[:], S[:])

    nc.compile()
    _BUILT[key] = nc
    return nc


def _core_inputs(core, positions, colors, opacities, scales, qvec, tvec,
                 intrinsics):
    b, xb = divmod(core, 4)
    R = _quat2mat(np.asarray(qvec, np.float32)[b])
    t = np.asarray(tvec, np.float32)[b]
    fx, fy, cx0, cy0 = np.asarray(intrinsics, np.float32)
    c = np.float32(SQ2I)

    pos = np.asarray(positions, np.float32)          # [N, 3]
    px = pos[:, 0].reshape(P, KC)
    py = pos[:, 1].reshape(P, KC)
    pz = pos[:, 2].reshape(P, KC)
    iss = np.float32(1.0) / np.asarray(scales, np.float32).reshape(P, KC)

    camx = px * R[0, 0] + py * R[0, 1] + pz * R[0, 2] + t[0]
    camy = px * R[1, 0] + py * R[1, 1] + pz * R[1, 2] + t[1]
    camz = px * R[2, 0] + py * R[2, 1] + pz * R[2, 2] + t[2]
    rz = np.float32(1.0) / camz

    isv = (c * iss).astype(np.float32)               # [P, KC]
    xs = (np.arange(NX, dtype=np.float32) + NX * xb - cx0)   # [NX]
    ys = (np.arange(NY, dtype=np.float32) - cy0)             # [NY]
    xi = np.empty((P, 1 + NX, KC), np.float32)
    xi[:, 0, :] = camx * (c * fx) * iss * rz         # ui
    xi[:, 1 : 1 + NX, :] = xs[None, :, None] * isv[:, None, :]

    opa4 = np.asarray(opacities, np.float32).reshape(P, KC) * np.float32(PI4)
    col = np.asarray(colors, np.float32)
    yu = np.empty((P, 12, KC), np.float32)
    vi = camy * (c * fy) * iss * rz                  # [P, KC]
    fd = ys[None, :, None] * isv[:, None, :] - vi[:, None, :]
    yu[:, 0:NY, :] = np.float32(2.0 / np.sqrt(np.pi)) * np.exp(
        -(fd.astype(np.float32) ** 2))
    for i in range(3):
        yu[:, NY + i, :] = opa4 * col[:, i].reshape(P, KC)
    yu[:, NY + 3, :] = opa4

    return {"yu": yu.astype(np.float16), "xi": xi.astype(np.float16)}


def kernel(positions, colors, opacities, scales, qvec, tvec, intrinsics,
           tile_hw, chunk_gauss, **run_kwargs):
    from concourse.bass_utils import run_bass_kernel_spmd

    tile_hw = int(tile_hw)
    chunk_gauss = int(chunk_gauss)
    assert tile_hw == 8 and positions.shape[0] == N_GAUSS
    n_chunks = -(-N_GAUSS // chunk_gauss)
    eps = np.float32(n_chunks * 1e-8)

    nc = _build()
    in_maps = [
        _core_inputs(c, positions, colors, opacities, scales, qvec, tvec,
                     intrinsics)
        for c in range(N_CORES)
    ]
    res = run_bass_kernel_spmd(nc, in_maps, core_ids=list(range(N_CORES)),
                               **run_kwargs)

    B = np.asarray(qvec).shape[0]
    img = np.zeros((B, 3, NY, 128), np.float32)
    for c in range(N_CORES):
        b, xb = divmod(c, 4)
        o = res.results[c]["out"]               # [4*32 (g,x), 2*32 (b,(d,y))]
        m = (o[:, 0:32] + o[:, 32:64]).reshape(4, NX, 32).sum(axis=0)
        num = m[:, 0:24].T.reshape(3, NY, NX)
        den = m[:, 24:32].T + eps               # [NY, NX]
        img[b, :, :, xb * NX : (xb + 1) * NX] = num / np.maximum(den, 1e-8)
    out = img.reshape(B, 3, NY * 128).reshape(B, 3, 128, 8)
    kernel.last_results = res
    return out


# revision 31
# speedup vs baseline: 1.0378x; 1.0378x over previous
"""Gaussian-splat differentiable renderer on 8 TRN2 NeuronCores.

The reference renders N=4096 isotropic 2D gaussians into a 128x128 image
but returns only the first 1024 pixels (y in [0,8), x in [0,128)) per
batch.  The gaussians are isotropic and pixels live on a grid, so the
weight separates: w[n,(x,y)] = g(n,x) * f(n,y), g = exp(-((x-u)*sd)^2),
f = exp(-((y-v)*sd)^2), sd = sqrt(0.5)/scale.

Sharding: 8 cores = batch (2) x x-blocks of 32 columns (4).  Each core
holds all N gaussians (partition p, chunk k; n = p*32+k) and owns its 32
x-columns end to end -- no collectives.  Host prep folds all O(N)
per-gaussian linear algebra (camera transform, projection, footprint
scaling) into the input layout; the device does the O(N x pixels)
render: subtract grids, exponentiate, weight-by-color, and the 4M-MAC
num/den accumulation per core.

Device-side structure (v8):
 - the g-side exp(-d^2) (the N x W bulk) via the ACT engine's
   Derivative_Erf table (d/dx erf = 2/sqrt(pi) e^{-x^2}); the 4/pi
   product factor is folded into the opacity-premultiplied colors
   host-side, and the small f-side factor (N x 8) rides in with the
   host prep.  DVE runs only stock fp16 SUBTRACTs (2 elem/cycle mode),
   with the gaussian-chunk axis k LAST so broadcasts land on middle
   dims.
 - two DMAs, one per HWDGE ring: sync carries ui+XI (everything the
   g path and hence the ACT chain needs); scalar carries EFH+OC
   (needed only by T3, which has slack behind the ACT table loads).
 - the 32 PSUM-accumulated fp16 matmuls rotate the stationary operand
   across the four 32-column PE groups (tile_position) so each chunk's
   LDWEIGHTS overlaps the previous chunk's MATMUL in a different
   sub-array; the four partial accumulators land in four partition
   blocks of one PSUM tile.
 - raw 4x(num|den) goes straight out (ACT copies PSUM->SBUF); the
   group-merge and final division happen host-side during unsharding.
 - ~115 dummy matmuls warm the PE clock (HAM) before the real stream.
"""

import numpy as np

N_GAUSS = 4096
P = 128          # partitions
KC = 32          # gaussian chunks along the free axis (n = p*KC + k)
KH = 16          # half of KC (ACT/matmul pipeline granularity)
NX = 32          # x columns per core
NY = 8           # y rows in the output
N_CORES = 8
SQ2I = 0.7071067811865476
PI4 = 0.7853981633974483   # pi/4, cancels the (2/sqrt(pi))^2 of D_Erf^2
N_WARM_MM = 115

_BUILT = {}


def _quat2mat(q):
    q = q.astype(np.float32)
    q = q / np.float32(np.sqrt(np.float32((q * q).sum())))
    w, x, y, z = [np.float32(v) for v in q]
    return np.array(
        [
            [1 - 2 * (y * y + z * z), 2 * (x * y - z * w), 2 * (x * z + y * w)],
            [2 * (x * y + z * w), 1 - 2 * (x * x + z * z), 2 * (y * z - x * w)],
            [2 * (x * z - y * w), 2 * (y * z + x * w), 1 - 2 * (x * x + y * y)],
        ],
        np.float32,
    )


def _build():
    key = "nc"
    if key in _BUILT:
        return _BUILT[key]

    import concourse.mybir as mybir
    import concourse.tile as tile
    from concourse import bacc
    from concourse.tile_rust import add_dep_helper

    f32 = mybir.dt.float32
    f16 = mybir.dt.float16
    DERF = mybir.ActivationFunctionType.Derivative_Erf
    COPY = mybir.ActivationFunctionType.Copy

    nc = bacc.Bacc("TRN2", target_bir_lowering=False, debug=False,
                   enable_asserts=False, num_devices=N_CORES)

    # rows 0-7: EFH[y,k] = (2/sqrt(pi))*exp(-(YI-vi)^2);
    # rows 8-11: OC' = (pi/4)*opa*(r,g,b,1)
    yu_d = nc.dram_tensor("yu", [P, 12, KC], f16, kind="ExternalInput")
    # row 0: ui (projected x center in sd units);
    # rows 1-32: XI[p, x, k] = (x + 32*xb - cx)*c*iss[p,k]   (k LAST)
    xi_d = nc.dram_tensor("xi", [P, 1 + NX, KC], f16, kind="ExternalInput")
    # rows 32g+x: group-g partial; cols: (d,y) d-major, 24 num + 8 den
    out_d = nc.dram_tensor("out", [P, 32], f32, kind="ExternalOutput")

    with tile.TileContext(nc) as tc:
        with (
            tc.tile_pool(name="sb", bufs=1) as pool,
            tc.tile_pool(name="ps", bufs=1, space="PSUM") as psum,
        ):
            # PE warm-up: independent matmuls keep the PE activity window
            # hot so the real stream runs at 2.4 GHz.
            DW = pool.tile([P, NX], f16)
            PSD = psum.tile([NX, NX], f32)
            nc.gpsimd.memset(DW[:], 0.25)
            for _ in range(N_WARM_MM):
                nc.tensor.matmul(PSD[:], DW[:], DW[:], start=True, stop=True)

            YU = pool.tile([P, 12, KC], f16)
            XU = pool.tile([P, 1 + NX, KC], f16)
            # one DMA per HWDGE ring; the whole g path (which gates the
            # ACT chain) depends only on the sync-ring tensor, while the
            # scalar-ring tensor is needed only by T3 (which has slack)
            nc.sync.dma_start(XU[:], xi_d[:])
            nc.scalar.dma_start(YU[:], yu_d[:])

            EFH = YU[:, 0:NY, :]
            OC = YU[:, NY : NY + 4, :]

            EGA = pool.tile([P, NX, KC], f16)
            EGH = pool.tile([P, NX, KC], f16)
            T3 = pool.tile([P, 4, NY, KC], f16)
            PS = psum.tile([P, 32], f32)

            # g path halves: d = XI - UI (k-last keeps broadcasts mid-dim)
            g_subs = []
            for s in range(2):
                ks = slice(s * KH, (s + 1) * KH)
                g_subs.append(nc.vector.tensor_sub(
                    EGA[:, :, ks], XU[:, 1 : 1 + NX, ks],
                    XU[:, 0, None, ks].broadcast_to([P, NX, KH])))

            for s in range(2):
                ks = slice(s * KH, (s + 1) * KH)
                nc.scalar.activation(EGH[:, :, ks], EGA[:, :, ks], DERF)

            # T3[p, d, y, k] = EFH[p, y, k] * OC[p, d, k], on DVE (2x);
            # scheduling-only dep keeps the subs ahead of T3 on the DVE
            # queue even if the model mispredicts the two DMA latencies
            for s in range(2):
                ks = slice(s * KH, (s + 1) * KH)
                t3_op = nc.vector.tensor_mul(
                    T3[:, :, :, ks],
                    EFH[:, None, :, ks].broadcast_to([P, 4, NY, KH]),
                    OC[:, :, None, ks].broadcast_to([P, 4, NY, KH]),
                )
                add_dep_helper(t3_op.ins, g_subs[1].ins, sync=False,
                               reason="subs before T3 on DVE")

            # rotate chunks across the 4 PE column groups: LDW(k+1)
            # overlaps MM(k) in a different 32x32 sub-array column strip
            for k in range(KC):
                g = k & 3
                nc.tensor.matmul(
                    PS[32 * g : 32 * (g + 1), :], EGH[:, :, k],
                    T3[:, :, :, k].rearrange("x a b -> x (a b)"),
                    start=(k < 4), stop=(k >= KC - 4),
                    tile_position=(0, 32 * g),
                )

            # raw group partials to DRAM (merge + division on the host);
            # the ACT engine is idle and reads PSUM quickly
            S = pool.tile([P, 32], f32)
            nc.scalar.activation(S[:], PS[:], COPY)
            nc.sync.dma_start(out_d[:], S[:])

    nc.compile()
    _BUILT[key] = nc
    return nc


def _core_inputs(core, positions, colors, opacities, scales, qvec, tvec,
                 intrinsics):
    b, xb = divmod(core, 4)
    R = _quat2mat(np.asarray(qvec, np.float32)[b])
    t = np.asarray(tvec, np.float32)[b]
    fx, fy, cx0, cy0 = np.asarray(intrinsics, np.float32)
    c = np.float32(SQ2I)

    pos = np.asarray(positions, np.float32)          # [N, 3]
    px = pos[:, 0].reshape(P, KC)
    py = pos[:, 1].reshape(P, KC)
    pz = pos[:, 2].reshape(P, KC)
    iss = np.float32(1.0) / np.asarray(scales, np.float32).reshape(P, KC)

    camx = px * R[0, 0] + py * R[0, 1] + pz * R[0, 2] + t[0]
    camy = px * R[1, 0] + py * R[1, 1] + pz * R[1, 2] + t[1]
    camz = px * R[2, 0] + py * R[2, 1] + pz * R[2, 2] + t[2]
    rz = np.float32(1.0) / camz

    isv = (c * iss).astype(np.float32)               # [P, KC]
    xs = (np.arange(NX, dtype=np.float32) + NX * xb - cx0)   # [NX]
    ys = (np.arange(NY, dtype=np.float32) - cy0)             # [NY]
    xi = np.empty((P, 1 + NX, KC), np.float32)
    xi[:, 0, :] = camx * (c * fx) * iss * rz         # ui
    xi[:, 1 : 1 + NX, :] = xs[None, :, None] * isv[:, None, :]

    opa4 = np.asarray(opacities, np.float32).reshape(P, KC) * np.float32(PI4)
    col = np.asarray(colors, np.float32)
    yu = np.empty((P, 12, KC), np.float32)
    vi = camy * (c * fy) * iss * rz                  # [P, KC]
    fd = ys[None, :, None] * isv[:, None, :] - vi[:, None, :]
    yu[:, 0:NY, :] = np.float32(2.0 / np.sqrt(np.pi)) * np.exp(
        -(fd.astype(np.float32) ** 2))
    for i in range(3):
        yu[:, NY + i, :] = opa4 * col[:, i].reshape(P, KC)
    yu[:, NY + 3, :] = opa4

    return {"yu": yu.astype(np.float16), "xi": xi.astype(np.float16)}


def kernel(positions, colors, opacities, scales, qvec, tvec, intrinsics,
           tile_hw, chunk_gauss, **run_kwargs):
    from concourse.bass_utils import run_bass_kernel_spmd

    tile_hw = int(tile_hw)
    chunk_gauss = int(chunk_gauss)
    assert tile_hw == 8 and positions.shape[0] == N_GAUSS
    n_chunks = -(-N_GAUSS // chunk_gauss)
    eps = np.float32(n_chunks * 1e-8)

    nc = _build()
    in_maps = [
        _core_inputs(c, positions, colors, opacities, scales, qvec, tvec,
                     intrinsics)
        for c in range(N_CORES)
    ]
    res = run_bass_kernel_spmd(nc, in_maps, core_ids=list(range(N_CORES)),
                               **run_kwargs)

    B = np.asarray(qvec).shape[0]
    img = np.zeros((B, 3, NY, 128), np.float32)
    for c in range(N_CORES):
        b, xb = divmod(c, 4)
        o = res.results[c]["out"]               # [4*32 (g,x), 32 (d*8+y)]
        m = o.reshape(4, NX, 32).sum(axis=0)    # [32 x, 32 (d,y)]
        num = m[:, 0:24].T.reshape(3, NY, NX)
        den = m[:, 24:32].T + eps               # [NY, NX]
        img[b, :, :, xb * NX : (xb + 1) * NX] = num / np.maximum(den, 1e-8)
    out = img.reshape(B, 3, NY * 128).reshape(B, 3, 128, 8)
    kernel.last_results = res
    return out


# revision 32
# speedup vs baseline: 1.0388x; 1.0009x over previous
"""Gaussian-splat differentiable renderer on 8 TRN2 NeuronCores.

The reference renders N=4096 isotropic 2D gaussians into a 128x128 image
but returns only the first 1024 pixels (y in [0,8), x in [0,128)) per
batch.  The gaussians are isotropic and pixels live on a grid, so the
weight separates: w[n,(x,y)] = g(n,x) * f(n,y), g = exp(-((x-u)*sd)^2),
f = exp(-((y-v)*sd)^2), sd = sqrt(0.5)/scale.

Sharding: 8 cores = batch (2) x x-blocks of 32 columns (4).  Each core
holds all N gaussians (partition p, chunk k; n = p*32+k) and owns its 32
x-columns end to end -- no collectives.  Host prep folds all O(N)
per-gaussian linear algebra (camera transform, projection, footprint
scaling) into the input layout; the device does the O(N x pixels)
render: subtract grids, exponentiate, weight-by-color, and the 4M-MAC
num/den accumulation per core.

Device-side structure (final):
 - the g-side exp(-d^2) (the N x W bulk) via the ACT engine's
   Derivative_Erf table (d/dx erf = 2/sqrt(pi) e^{-x^2}); the 4/pi
   product factor is folded into the opacity-premultiplied colors
   host-side, and the small f-side factor (N x 8) rides in with the
   host prep.  DVE runs only stock fp16 SUBTRACTs (2 elem/cycle mode),
   with the gaussian-chunk axis k LAST so broadcasts land on middle
   dims.
 - two DMAs, one per HWDGE ring: sync carries ui+XI (everything the
   g path and hence the ACT chain needs); scalar carries EFH+OC
   (needed only by T3, which has slack behind the ACT table loads).
 - the 32 PSUM-accumulated fp16 matmuls rotate the stationary operand
   across the four 32-column PE groups (tile_position) so each chunk's
   LDWEIGHTS overlaps the previous chunk's MATMUL in a different
   sub-array; the four partial accumulators land in four partition
   blocks of one PSUM tile.
 - raw 4x(num|den) goes straight out (ACT copies PSUM->SBUF); the
   group-merge and final division happen host-side during unsharding.
 - ~115 dummy matmuls warm the PE clock (HAM) before the real stream.
"""

import numpy as np

N_GAUSS = 4096
P = 128          # partitions
KC = 32          # gaussian chunks along the free axis (n = p*KC + k)
KH = 16          # half of KC (ACT/matmul pipeline granularity)
NX = 32          # x columns per core
NY = 8           # y rows in the output
N_CORES = 8
SQ2I = 0.7071067811865476
PI4 = 0.7853981633974483   # pi/4, cancels the (2/sqrt(pi))^2 of D_Erf^2
N_WARM_MM = 115

_BUILT = {}


def _quat2mat(q):
    q = q.astype(np.float32)
    q = q / np.float32(np.sqrt(np.float32((q * q).sum())))
    w, x, y, z = [np.float32(v) for v in q]
    return np.array(
        [
            [1 - 2 * (y * y + z * z), 2 * (x * y - z * w), 2 * (x * z + y * w)],
            [2 * (x * y + z * w), 1 - 2 * (x * x + z * z), 2 * (y * z - x * w)],
            [2 * (x * z - y * w), 2 * (y * z + x * w), 1 - 2 * (x * x + y * y)],
        ],
        np.float32,
    )


def _build():
    key = "nc"
    if key in _BUILT:
        return _BUILT[key]

    import concourse.mybir as mybir
    import concourse.tile as tile
    from concourse import bacc
    from concourse.tile_rust import add_dep_helper

    f32 = mybir.dt.float32
    f16 = mybir.dt.float16
    DERF = mybir.ActivationFunctionType.Derivative_Erf
    COPY = mybir.ActivationFunctionType.Copy

    nc = bacc.Bacc("TRN2", target_bir_lowering=False, debug=False,
                   enable_asserts=False, num_devices=N_CORES)

    # rows 0-7: EFH[y,k] = (2/sqrt(pi))*exp(-(YI-vi)^2);
    # rows 8-11: OC' = (pi/4)*opa*(r,g,b,1)
    yu_d = nc.dram_tensor("yu", [P, 12, KC], f16, kind="ExternalInput")
    # row 0: ui (projected x center in sd units);
    # rows 1-32: XI[p, x, k] = (x + 32*xb - cx)*c*iss[p,k]   (k LAST)
    xi_d = nc.dram_tensor("xi", [P, 1 + NX, KC], f16, kind="ExternalInput")
    # rows 32g+x: group-g partial; cols: (d,y) d-major, 24 num + 8 den
    out_d = nc.dram_tensor("out", [P, 32], f32, kind="ExternalOutput")

    with tile.TileContext(nc) as tc:
        with (
            tc.tile_pool(name="sb", bufs=1) as pool,
            tc.tile_pool(name="ps", bufs=1, space="PSUM") as psum,
        ):
            # PE warm-up: independent matmuls keep the PE activity window
            # hot so the real stream runs at 2.4 GHz.
            DW = pool.tile([P, NX], f16)
            PSD = psum.tile([NX, NX], f32)
            nc.gpsimd.memset(DW[:], 0.25)
            for _ in range(N_WARM_MM):
                nc.tensor.matmul(PSD[:], DW[:], DW[:], start=True, stop=True)

            YU = pool.tile([P, 12, KC], f16)
            XU = pool.tile([P, 1 + NX, KC], f16)
            # one DMA per HWDGE ring; the whole g path (which gates the
            # ACT chain) depends only on the sync-ring tensor, while the
            # scalar-ring tensor is needed only by T3 (which has slack)
            nc.sync.dma_start(XU[:], xi_d[:])
            nc.scalar.dma_start(YU[:], yu_d[:])

            EFH = YU[:, 0:NY, :]
            OC = YU[:, NY : NY + 4, :]

            EGA = pool.tile([P, NX, KC], f16)
            EGH = pool.tile([P, NX, KC], f16)
            T3 = pool.tile([P, 4, NY, KC], f16)
            PS = psum.tile([P, 32], f32)

            # g path halves: d = XI - UI (k-last keeps broadcasts mid-dim)
            g_subs = []
            for s in range(2):
                ks = slice(s * KH, (s + 1) * KH)
                g_subs.append(nc.vector.tensor_sub(
                    EGA[:, :, ks], XU[:, 1 : 1 + NX, ks],
                    XU[:, 0, None, ks].broadcast_to([P, NX, KH])))

            for s in range(2):
                ks = slice(s * KH, (s + 1) * KH)
                nc.scalar.activation(EGH[:, :, ks], EGA[:, :, ks], DERF)

            # T3[p, d, y, k] = EFH[p, y, k] * OC[p, d, k], on DVE (2x);
            # scheduling-only dep keeps the subs ahead of T3 on the DVE
            # queue even if the model mispredicts the two DMA latencies
            for s in range(2):
                ks = slice(s * KH, (s + 1) * KH)
                t3_op = nc.vector.tensor_mul(
                    T3[:, :, :, ks],
                    EFH[:, None, :, ks].broadcast_to([P, 4, NY, KH]),
                    OC[:, :, None, ks].broadcast_to([P, 4, NY, KH]),
                )
                add_dep_helper(t3_op.ins, g_subs[1].ins, sync=False,
                               reason="subs before T3 on DVE")

            # rotate chunks across the 4 PE column groups: LDW(k+1)
            # overlaps MM(k) in a different 32x32 sub-array column strip
            for k in range(KC):
                g = k & 3
                nc.tensor.matmul(
                    PS[32 * g : 32 * (g + 1), :], EGH[:, :, k],
                    T3[:, :, :, k].rearrange("x a b -> x (a b)"),
                    start=(k < 4), stop=(k >= KC - 4),
                    tile_position=(0, 32 * g),
                )

            # raw group partials to DRAM (merge + division on the host);
            # the ACT engine is idle and reads PSUM quickly
            S = pool.tile([P, 32], f32)
            nc.scalar.activation(S[:], PS[:], COPY)
            nc.sync.dma_start(out_d[:], S[:])

    nc.compile()
    _BUILT[key] = nc
    return nc


def _core_inputs(core, positions, colors, opacities, scales, qvec, tvec,
                 intrinsics):
    b, xb = divmod(core, 4)
    R = _quat2mat(np.asarray(qvec, np.float32)[b])
    t = np.asarray(tvec, np.float32)[b]
    fx, fy, cx0, cy0 = np.asarray(intrinsics, np.float32)
    c = np.float32(SQ2I)

    pos = np.asarray(positions, np.float32)          # [N, 3]
    px = pos[:, 0].reshape(P, KC)
    py = pos[:, 1].reshape(P, KC)
    pz = pos[:, 2].reshape(P, KC)
    iss = np.float32(1.0) / np.asarray(scales, np.float32).reshape(P, KC)

    camx = px * R[0, 0] + py * R[0, 1] + pz * R[0, 2] + t[0]
    camy = px * R[1, 0] + py * R[1, 1] + pz * R[1, 2] + t[1]
    camz = px * R[2, 0] + py * R[2, 1] + pz * R[2, 2] + t[2]
    rz = np.float32(1.0) / camz

    isv = (c * iss).astype(np.float32)               # [P, KC]
    xs = (np.arange(NX, dtype=np.float32) + NX * xb - cx0)   # [NX]
    ys = (np.arange(NY, dtype=np.float32) - cy0)             # [NY]
    xi = np.empty((P, 1 + NX, KC), np.float32)
    xi[:, 0, :] = camx * (c * fx) * iss * rz         # ui
    xi[:, 1 : 1 + NX, :] = xs[None, :, None] * isv[:, None, :]

    opa4 = np.asarray(opacities, np.float32).reshape(P, KC) * np.float32(PI4)
    col = np.asarray(colors, np.float32)
    yu = np.empty((P, 12, KC), np.float32)
    vi = camy * (c * fy) * iss * rz                  # [P, KC]
    fd = ys[None, :, None] * isv[:, None, :] - vi[:, None, :]
    yu[:, 0:NY, :] = np.float32(2.0 / np.sqrt(np.pi)) * np.exp(
        -(fd.astype(np.float32) ** 2))
    for i in range(3):
        yu[:, NY + i, :] = opa4 * col[:, i].reshape(P, KC)
    yu[:, NY + 3, :] = opa4

    return {"yu": yu.astype(np.float16), "xi": xi.astype(np.float16)}


def kernel(positions, colors, opacities, scales, qvec, tvec, intrinsics,
           tile_hw, chunk_gauss, **run_kwargs):
    from concourse.bass_utils import run_bass_kernel_spmd

    tile_hw = int(tile_hw)
    chunk_gauss = int(chunk_gauss)
    assert tile_hw == 8 and positions.shape[0] == N_GAUSS
    n_chunks = -(-N_GAUSS // chunk_gauss)
    eps = np.float32(n_chunks * 1e-8)

    nc = _build()
    in_maps = [
        _core_inputs(c, positions, colors, opacities, scales, qvec, tvec,
                     intrinsics)
        for c in range(N_CORES)
    ]
    res = run_bass_kernel_spmd(nc, in_maps, core_ids=list(range(N_CORES)),
                               **run_kwargs)

    B = np.asarray(qvec).shape[0]
    img = np.zeros((B, 3, NY, 128), np.float32)
    for c in range(N_CORES):
        b, xb = divmod(c, 4)
        o = res.results[c]["out"]               # [4*32 (g,x), 32 (d*8+y)]
        m = o.reshape(4, NX, 32).sum(axis=0)    # [32 x, 32 (d,y)]
        num = m[:, 0:24].T.reshape(3, NY, NX)
        den = m[:, 24:32].T + eps               # [NY, NX]
        img[b, :, :, xb * NX : (xb + 1) * NX] = num / np.maximum(den, 1e-8)
    out = img.reshape(B, 3, NY * 128).reshape(B, 3, 128, 8)
    kernel.last_results = res
    return out


# revision 39
# speedup vs baseline: 1.0706x; 1.0307x over previous
"""Gaussian-splat differentiable renderer on 8 TRN2 NeuronCores.

The reference renders N=4096 isotropic 2D gaussians into a 128x128 image
but returns only the first 1024 pixels (y in [0,8), x in [0,128)) per
batch.  The gaussians are isotropic and pixels live on a grid, so the
weight separates: w[n,(x,y)] = g(n,x) * f(n,y), g = exp(-((x-u)*sd)^2),
f = exp(-((y-v)*sd)^2), sd = sqrt(0.5)/scale.

Sharding: 8 cores = batch (2) x x-blocks of 32 columns (4).  Each core
holds all N gaussians (partition p, chunk k; n = p*32+k) and owns its 32
x-columns end to end -- no collectives.  Host prep folds all O(N)
per-gaussian linear algebra (camera transform, projection, footprint
scaling) into the input layout; the device does the O(N x pixels)
render: subtract grids, exponentiate, weight-by-color, and the 4M-MAC
num/den accumulation per core.

Device-side structure (final):
 - the g-side exp(-d^2) (the N x W bulk) via the ACT engine's
   Derivative_Erf table (d/dx erf = 2/sqrt(pi) e^{-x^2}); the 4/pi
   product factor is folded into the opacity-premultiplied colors
   host-side, and the small f-side factor (N x 8) rides in with the
   host prep.  DVE runs only stock fp16 SUBTRACTs (2 elem/cycle mode),
   with the gaussian-chunk axis k LAST so broadcasts land on middle
   dims.
 - two DMAs, one per HWDGE ring: sync carries a tiny ui/isv/offset
   tensor (everything the g path and hence the ACT chain needs — the
   pixel grid itself is synthesized by a custom DVE op from the page
   index); scalar carries EFH+OC (needed only by T3, which has slack
   behind the ACT table loads).
 - the 32 PSUM-accumulated fp16 matmuls rotate the stationary operand
   across the four 32-column PE groups (tile_position) so each chunk's
   LDWEIGHTS overlaps the previous chunk's MATMUL in a different
   sub-array; the four partial accumulators land in four partition
   blocks of one PSUM tile.
 - raw 4x(num|den) goes straight out (ACT copies PSUM->SBUF); the
   group-merge and final division happen host-side during unsharding.
 - ~115 dummy matmuls warm the PE clock (HAM) before the real stream.
"""

import numpy as np

N_GAUSS = 4096
P = 128          # partitions
KC = 32          # gaussian chunks along the free axis (n = p*KC + k)
KH = 16          # half of KC (ACT/matmul pipeline granularity)
NX = 32          # x columns per core
NY = 8           # y rows in the output
N_CORES = 8
SQ2I = 0.7071067811865476
PI4 = 0.7853981633974483   # pi/4, cancels the (2/sqrt(pi))^2 of D_Erf^2
N_WARM_MM = 115

_BUILT = {}


def _quat2mat(q):
    q = q.astype(np.float32)
    q = q / np.float32(np.sqrt(np.float32((q * q).sum())))
    w, x, y, z = [np.float32(v) for v in q]
    return np.array(
        [
            [1 - 2 * (y * y + z * z), 2 * (x * y - z * w), 2 * (x * z + y * w)],
            [2 * (x * y + z * w), 1 - 2 * (x * x + z * z), 2 * (y * z - x * w)],
            [2 * (x * z - y * w), 2 * (y * z + x * w), 1 - 2 * (x * x + y * y)],
        ],
        np.float32,
    )


def _register_axpb_sub():
    """Register a custom DVE op out = (page_idx + s0) * in0 - in1 (the
    sanctioned extension point: append to dve_ops.OPS).  With pages = x
    and elements = k, this synthesizes the pixel grid XI = xs*isv on the
    fly, so only isv/ui/offset rows need to come over the DMA."""
    import numpy as np
    import concourse.dve_ops as dvo
    from concourse.dve_spec import Spec, Src0, Src1, SubIdx, C0, lower
    from concourse.dve_uop import DveOpSpec

    for op in dvo.OPS:
        if op.name == "AXPB_SUB_ANT":
            return op
    spec = Spec(
        body=(SubIdx + C0) * Src0 - Src1,
        reference=lambda in0, in1, s0, s1, imm2: (
            (np.arange(in0.shape[1], dtype=np.float32)[None, :, None]
             + s0[:, None, None]) * in0.astype(np.float32)
            - in1.astype(np.float32)
        ),
    )
    op = dvo.DveOp("AXPB_SUB_ANT", spec, subdim=True, uops_sha={})
    dvo.OPS.append(op)
    dvo._SUB_OPCODE_FOR_NAME[op.name] = dvo._CUSTOM_DVE_ROW_BASE + len(dvo.OPS) - 1
    dvo.CUSTOM_DVE_SPECS[op.name] = spec
    for ver in ("v3", "v4"):
        s = DveOpSpec(
            name=op.name,
            opcode=dvo.get_dve_sub_opcode(op.name),
            uops=lower(spec, ver=ver),
            rd1_en=True,
        )
        op.uops_sha[ver] = s.sha(ver)
    return op


def _build():
    key = "nc"
    if key in _BUILT:
        return _BUILT[key]

    import concourse.mybir as mybir
    import concourse.tile as tile
    from concourse import bacc
    from concourse.tile_rust import add_dep_helper

    SUBX = _register_axpb_sub()

    f32 = mybir.dt.float32
    f16 = mybir.dt.float16
    DERF = mybir.ActivationFunctionType.Derivative_Erf
    COPY = mybir.ActivationFunctionType.Copy

    nc = bacc.Bacc("TRN2", target_bir_lowering=False, debug=False,
                   enable_asserts=False, num_devices=N_CORES)

    # rows 0-7: EFH[y,k] = (2/sqrt(pi))*exp(-(YI-vi)^2);
    # rows 8-11: OC' = (pi/4)*opa*(r,g,b,1)
    yu_d = nc.dram_tensor("yu", [P, 12, KC], f16, kind="ExternalInput")
    # rows: 0 ui (projected x center in sd units), 1 isv = c*iss,
    # 2 xoff = 32*xb - cx broadcast (the per-core pixel-grid offset)
    xu_d = nc.dram_tensor("xu", [P, 3, KC], f32, kind="ExternalInput")
    # rows 32g+x: group-g partial; cols: (d,y) d-major, 24 num + 8 den
    out_d = nc.dram_tensor("out", [P, 32], f32, kind="ExternalOutput")

    with tile.TileContext(nc) as tc:
        with (
            tc.tile_pool(name="sb", bufs=1) as pool,
            tc.tile_pool(name="ps", bufs=1, space="PSUM") as psum,
        ):
            # PE warm-up: independent matmuls keep the PE activity window
            # hot so the real stream runs at 2.4 GHz.
            DW = pool.tile([P, NX], f16)
            PSD = psum.tile([NX, NX], f32)
            nc.gpsimd.memset(DW[:], 0.25)
            for _ in range(N_WARM_MM):
                nc.tensor.matmul(PSD[:], DW[:], DW[:], start=True, stop=True)

            YU = pool.tile([P, 12, KC], f16)
            XU = pool.tile([P, 3, KC], f32)
            # one DMA per HWDGE ring; the whole g path (which gates the
            # ACT chain) depends only on the tiny sync-ring tensor, while
            # the scalar-ring tensor is needed only by T3 (which has slack)
            nc.sync.dma_start(XU[:], xu_d[:])
            nc.scalar.dma_start(YU[:], yu_d[:])

            EFH = YU[:, 0:NY, :]
            OC = YU[:, NY : NY + 4, :]

            EGA = pool.tile([P, NX, KC], f16)
            EGH = pool.tile([P, NX, KC], f16)
            T3 = pool.tile([P, 4, NY, KC], f16)
            PS = psum.tile([P, 32], f32)

            # g path halves: d = (x + xoff)*isv - ui, with the pixel
            # coordinate synthesized by the DVE's page index (SubIdx)
            g_subs = []
            for s in range(2):
                ks = slice(s * KH, (s + 1) * KH)
                g_subs.append(nc.vector._custom_dve(
                    SUBX, out=EGA[:, :, ks],
                    in0=XU[:, 1, None, ks].broadcast_to([P, NX, KH]),
                    in1=XU[:, 0, None, ks].broadcast_to([P, NX, KH]),
                    s0=XU[:, 2, 0:1]))

            for s in range(2):
                ks = slice(s * KH, (s + 1) * KH)
                nc.scalar.activation(EGH[:, :, ks], EGA[:, :, ks], DERF)

            # T3[p, d, y, k] = EFH[p, y, k] * OC[p, d, k], on DVE (2x);
            # scheduling-only dep keeps the subs ahead of T3 on the DVE
            # queue even if the model mispredicts the two DMA latencies
            for s in range(2):
                ks = slice(s * KH, (s + 1) * KH)
                t3_op = nc.vector.tensor_mul(
                    T3[:, :, :, ks],
                    EFH[:, None, :, ks].broadcast_to([P, 4, NY, KH]),
                    OC[:, :, None, ks].broadcast_to([P, 4, NY, KH]),
                )
                add_dep_helper(t3_op.ins, g_subs[1].ins, sync=False,
                               reason="subs before T3 on DVE")

            # rotate chunks across the 4 PE column groups: LDW(k+1)
            # overlaps MM(k) in a different 32x32 sub-array column strip
            for k in range(KC):
                g = k & 3
                nc.tensor.matmul(
                    PS[32 * g : 32 * (g + 1), :], EGH[:, :, k],
                    T3[:, :, :, k].rearrange("x a b -> x (a b)"),
                    start=(k < 4), stop=(k >= KC - 4),
                    tile_position=(0, 32 * g),
                )

            # raw group partials to DRAM (merge + division on the host);
            # the ACT engine is idle and reads PSUM quickly
            S = pool.tile([P, 32], f32)
            nc.scalar.activation(S[:], PS[:], COPY)
            nc.sync.dma_start(out_d[:], S[:])

    nc.compile()
    _BUILT[key] = nc
    return nc


def _core_inputs(core, positions, colors, opacities, scales, qvec, tvec,
                 intrinsics):
    b, xb = divmod(core, 4)
    R = _quat2mat(np.asarray(qvec, np.float32)[b])
    t = np.asarray(tvec, np.float32)[b]
    fx, fy, cx0, cy0 = np.asarray(intrinsics, np.float32)
    c = np.float32(SQ2I)

    pos = np.asarray(positions, np.float32)          # [N, 3]
    px = pos[:, 0].reshape(P, KC)
    py = pos[:, 1].reshape(P, KC)
    pz = pos[:, 2].reshape(P, KC)
    iss = np.float32(1.0) / np.asarray(scales, np.float32).reshape(P, KC)

    camx = px * R[0, 0] + py * R[0, 1] + pz * R[0, 2] + t[0]
    camy = px * R[1, 0] + py * R[1, 1] + pz * R[1, 2] + t[1]
    camz = px * R[2, 0] + py * R[2, 1] + pz * R[2, 2] + t[2]
    rz = np.float32(1.0) / camz

    isv = (c * iss).astype(np.float32)               # [P, KC]
    ys = (np.arange(NY, dtype=np.float32) - cy0)             # [NY]
    xu = np.empty((P, 3, KC), np.float32)
    xu[:, 0, :] = camx * (c * fx) * iss * rz         # ui
    xu[:, 1, :] = isv
    xu[:, 2, :] = np.float32(NX * xb) - cx0          # xoff

    opa4 = np.asarray(opacities, np.float32).reshape(P, KC) * np.float32(PI4)
    col = np.asarray(colors, np.float32)
    yu = np.empty((P, 12, KC), np.float32)
    vi = camy * (c * fy) * iss * rz                  # [P, KC]
    fd = ys[None, :, None] * isv[:, None, :] - vi[:, None, :]
    yu[:, 0:NY, :] = np.float32(2.0 / np.sqrt(np.pi)) * np.exp(
        -(fd.astype(np.float32) ** 2))
    for i in range(3):
        yu[:, NY + i, :] = opa4 * col[:, i].reshape(P, KC)
    yu[:, NY + 3, :] = opa4

    return {"yu": yu.astype(np.float16), "xu": xu}


def kernel(positions, colors, opacities, scales, qvec, tvec, intrinsics,
           tile_hw, chunk_gauss, **run_kwargs):
    from concourse.bass_utils import run_bass_kernel_spmd

    tile_hw = int(tile_hw)
    chunk_gauss = int(chunk_gauss)
    assert tile_hw == 8 and positions.shape[0] == N_GAUSS
    n_chunks = -(-N_GAUSS // chunk_gauss)
    eps = np.float32(n_chunks * 1e-8)

    nc = _build()
    in_maps = [
        _core_inputs(c, positions, colors, opacities, scales, qvec, tvec,
                     intrinsics)
        for c in range(N_CORES)
    ]
    res = run_bass_kernel_spmd(nc, in_maps, core_ids=list(range(N_CORES)),
                               **run_kwargs)

    B = np.asarray(qvec).shape[0]
    img = np.zeros((B, 3, NY, 128), np.float32)
    for c in range(N_CORES):
        b, xb = divmod(c, 4)
        o = res.results[c]["out"]               # [4*32 (g,x), 32 (d*8+y)]
        m = o.reshape(4, NX, 32).sum(axis=0)    # [32 x, 32 (d,y)]
        num = m[:, 0:24].T.reshape(3, NY, NX)
        den = m[:, 24:32].T + eps               # [NY, NX]
        img[b, :, :, xb * NX : (xb + 1) * NX] = num / np.maximum(den, 1e-8)
    out = img.reshape(B, 3, NY * 128).reshape(B, 3, 128, 8)
    kernel.last_results = res
    return out


# revision 40
# speedup vs baseline: 1.0710x; 1.0004x over previous
"""Gaussian-splat differentiable renderer on 8 TRN2 NeuronCores.

The reference renders N=4096 isotropic 2D gaussians into a 128x128 image
but returns only the first 1024 pixels (y in [0,8), x in [0,128)) per
batch.  The gaussians are isotropic and pixels live on a grid, so the
weight separates: w[n,(x,y)] = g(n,x) * f(n,y), g = exp(-((x-u)*sd)^2),
f = exp(-((y-v)*sd)^2), sd = sqrt(0.5)/scale.

Sharding: 8 cores = batch (2) x x-blocks of 32 columns (4).  Each core
holds all N gaussians (partition p, chunk k; n = p*32+k) and owns its 32
x-columns end to end -- no collectives.  Host prep folds all O(N)
per-gaussian linear algebra (camera transform, projection, footprint
scaling) into the input layout; the device does the O(N x pixels)
render: subtract grids, exponentiate, weight-by-color, and the 4M-MAC
num/den accumulation per core.

Device-side structure (final):
 - the g-side exp(-d^2) (the N x W bulk) via the ACT engine's
   Derivative_Erf table (d/dx erf = 2/sqrt(pi) e^{-x^2}); the 4/pi
   product factor is folded into the opacity-premultiplied colors
   host-side, and the small f-side factor (N x 8) rides in with the
   host prep.  DVE runs only stock fp16 SUBTRACTs (2 elem/cycle mode),
   with the gaussian-chunk axis k LAST so broadcasts land on middle
   dims.
 - two DMAs, one per HWDGE ring: sync carries a tiny ui/isv/offset
   tensor (everything the g path and hence the ACT chain needs — the
   pixel grid itself is synthesized by a custom DVE op from the page
   index); scalar carries EFH+OC (needed only by T3, which has slack
   behind the ACT table loads).
 - the 32 PSUM-accumulated fp16 matmuls rotate the stationary operand
   across the four 32-column PE groups (tile_position) so each chunk's
   LDWEIGHTS overlaps the previous chunk's MATMUL in a different
   sub-array; the four partial accumulators land in four partition
   blocks of one PSUM tile.
 - raw 4x(num|den) goes straight out (ACT copies PSUM->SBUF); the
   group-merge and final division happen host-side during unsharding.
 - ~115 dummy matmuls warm the PE clock (HAM) before the real stream.
"""

import numpy as np

N_GAUSS = 4096
P = 128          # partitions
KC = 32          # gaussian chunks along the free axis (n = p*KC + k)
KH = 16          # half of KC (ACT/matmul pipeline granularity)
NX = 32          # x columns per core
NY = 8           # y rows in the output
N_CORES = 8
SQ2I = 0.7071067811865476
PI4 = 0.7853981633974483   # pi/4, cancels the (2/sqrt(pi))^2 of D_Erf^2
N_WARM_MM = 115

_BUILT = {}


def _quat2mat(q):
    q = q.astype(np.float32)
    q = q / np.float32(np.sqrt(np.float32((q * q).sum())))
    w, x, y, z = [np.float32(v) for v in q]
    return np.array(
        [
            [1 - 2 * (y * y + z * z), 2 * (x * y - z * w), 2 * (x * z + y * w)],
            [2 * (x * y + z * w), 1 - 2 * (x * x + z * z), 2 * (y * z - x * w)],
            [2 * (x * z - y * w), 2 * (y * z + x * w), 1 - 2 * (x * x + y * y)],
        ],
        np.float32,
    )


def _register_axpb_sub():
    """Register a custom DVE op out = (page_idx + s0) * in0 - in1 (the
    sanctioned extension point: append to dve_ops.OPS).  With pages = x
    and elements = k, this synthesizes the pixel grid XI = xs*isv on the
    fly, so only isv/ui/offset rows need to come over the DMA."""
    import numpy as np
    import concourse.dve_ops as dvo
    from concourse.dve_spec import Spec, Src0, Src1, SubIdx, C0, lower
    from concourse.dve_uop import DveOpSpec

    for op in dvo.OPS:
        if op.name == "AXPB_SUB_ANT":
            return op
    spec = Spec(
        body=(SubIdx + C0) * Src0 - Src1,
        reference=lambda in0, in1, s0, s1, imm2: (
            (np.arange(in0.shape[1], dtype=np.float32)[None, :, None]
             + s0[:, None, None]) * in0.astype(np.float32)
            - in1.astype(np.float32)
        ),
    )
    op = dvo.DveOp("AXPB_SUB_ANT", spec, subdim=True, uops_sha={})
    dvo.OPS.append(op)
    dvo._SUB_OPCODE_FOR_NAME[op.name] = dvo._CUSTOM_DVE_ROW_BASE + len(dvo.OPS) - 1
    dvo.CUSTOM_DVE_SPECS[op.name] = spec
    for ver in ("v3", "v4"):
        s = DveOpSpec(
            name=op.name,
            opcode=dvo.get_dve_sub_opcode(op.name),
            uops=lower(spec, ver=ver),
            rd1_en=True,
        )
        op.uops_sha[ver] = s.sha(ver)
    return op


def _build():
    key = "nc"
    if key in _BUILT:
        return _BUILT[key]

    import concourse.mybir as mybir
    import concourse.tile as tile
    from concourse import bacc
    from concourse.tile_rust import add_dep_helper

    SUBX = _register_axpb_sub()

    f32 = mybir.dt.float32
    f16 = mybir.dt.float16
    DERF = mybir.ActivationFunctionType.Derivative_Erf
    COPY = mybir.ActivationFunctionType.Copy

    nc = bacc.Bacc("TRN2", target_bir_lowering=False, debug=False,
                   enable_asserts=False, num_devices=N_CORES)

    # rows 0-7: EFH[y,k] = (2/sqrt(pi))*exp(-(YI-vi)^2);
    # rows 8-11: OC' = (pi/4)*opa*(r,g,b,1)
    yu_d = nc.dram_tensor("yu", [P, 12, KC], f16, kind="ExternalInput")
    # rows: 0 ui (projected x center in sd units), 1 isv = c*iss,
    # 2 xoff = 32*xb - cx broadcast (the per-core pixel-grid offset)
    xu_d = nc.dram_tensor("xu", [P, 3, KC], f32, kind="ExternalInput")
    # rows 32g+x: group-g partial; cols: (d,y) d-major, 24 num + 8 den
    out_d = nc.dram_tensor("out", [P, 32], f32, kind="ExternalOutput")

    with tile.TileContext(nc) as tc:
        with (
            tc.tile_pool(name="sb", bufs=1) as pool,
            tc.tile_pool(name="ps", bufs=1, space="PSUM") as psum,
        ):
            # PE warm-up: independent matmuls keep the PE activity window
            # hot so the real stream runs at 2.4 GHz.
            DW = pool.tile([P, NX], f16)
            PSD = psum.tile([NX, NX], f32)
            nc.gpsimd.memset(DW[:], 0.25)
            for _ in range(N_WARM_MM):
                nc.tensor.matmul(PSD[:], DW[:], DW[:], start=True, stop=True)

            YU = pool.tile([P, 12, KC], f16)
            XU = pool.tile([P, 3, KC], f32)
            # one DMA per HWDGE ring; the whole g path (which gates the
            # ACT chain) depends only on the tiny sync-ring tensor, while
            # the scalar-ring tensor is needed only by T3 (which has slack)
            nc.sync.dma_start(XU[:], xu_d[:])
            nc.scalar.dma_start(YU[:], yu_d[:])

            EFH = YU[:, 0:NY, :]
            OC = YU[:, NY : NY + 4, :]

            EGA = pool.tile([P, NX, KC], f16)
            EGH = pool.tile([P, NX, KC], f16)
            T3 = pool.tile([P, 4, NY, KC], f16)
            PS = psum.tile([P, 32], f32)

            # g path halves: d = (x + xoff)*isv - ui, with the pixel
            # coordinate synthesized by the DVE's page index (SubIdx)
            g_subs = []
            for s in range(2):
                ks = slice(s * KH, (s + 1) * KH)
                g_subs.append(nc.vector._custom_dve(
                    SUBX, out=EGA[:, :, ks],
                    in0=XU[:, 1, None, ks].broadcast_to([P, NX, KH]),
                    in1=XU[:, 0, None, ks].broadcast_to([P, NX, KH]),
                    s0=XU[:, 2, 0:1]))

            for s in range(2):
                ks = slice(s * KH, (s + 1) * KH)
                nc.scalar.activation(EGH[:, :, ks], EGA[:, :, ks], DERF)

            # T3[p, d, y, k] = EFH[p, y, k] * OC[p, d, k], on DVE (2x);
            # scheduling-only dep keeps the subs ahead of T3 on the DVE
            # queue even if the model mispredicts the two DMA latencies
            for s in range(2):
                ks = slice(s * KH, (s + 1) * KH)
                t3_op = nc.vector.tensor_mul(
                    T3[:, :, :, ks],
                    EFH[:, None, :, ks].broadcast_to([P, 4, NY, KH]),
                    OC[:, :, None, ks].broadcast_to([P, 4, NY, KH]),
                )
                add_dep_helper(t3_op.ins, g_subs[1].ins, sync=False,
                               reason="subs before T3 on DVE")

            # rotate chunks across the 4 PE column groups: LDW(k+1)
            # overlaps MM(k) in a different 32x32 sub-array column strip
            for k in range(KC):
                g = k & 3
                nc.tensor.matmul(
                    PS[32 * g : 32 * (g + 1), :], EGH[:, :, k],
                    T3[:, :, :, k].rearrange("x a b -> x (a b)"),
                    start=(k < 4), stop=(k >= KC - 4),
                    tile_position=(0, 32 * g),
                )

            # raw group partials to DRAM (merge + division on the host);
            # the DVE is idle here and its PSUM access latency (120cyc)
            # beats the ACT engine's (172cyc + table-engine overhead)
            S = pool.tile([P, 32], f32)
            nc.vector.tensor_copy(S[:], PS[:])
            nc.sync.dma_start(out_d[:], S[:])

    nc.compile()
    _BUILT[key] = nc
    return nc


def _core_inputs(core, positions, colors, opacities, scales, qvec, tvec,
                 intrinsics):
    b, xb = divmod(core, 4)
    R = _quat2mat(np.asarray(qvec, np.float32)[b])
    t = np.asarray(tvec, np.float32)[b]
    fx, fy, cx0, cy0 = np.asarray(intrinsics, np.float32)
    c = np.float32(SQ2I)

    pos = np.asarray(positions, np.float32)          # [N, 3]
    px = pos[:, 0].reshape(P, KC)
    py = pos[:, 1].reshape(P, KC)
    pz = pos[:, 2].reshape(P, KC)
    iss = np.float32(1.0) / np.asarray(scales, np.float32).reshape(P, KC)

    camx = px * R[0, 0] + py * R[0, 1] + pz * R[0, 2] + t[0]
    camy = px * R[1, 0] + py * R[1, 1] + pz * R[1, 2] + t[1]
    camz = px * R[2, 0] + py * R[2, 1] + pz * R[2, 2] + t[2]
    rz = np.float32(1.0) / camz

    isv = (c * iss).astype(np.float32)               # [P, KC]
    ys = (np.arange(NY, dtype=np.float32) - cy0)             # [NY]
    xu = np.empty((P, 3, KC), np.float32)
    xu[:, 0, :] = camx * (c * fx) * iss * rz         # ui
    xu[:, 1, :] = isv
    xu[:, 2, :] = np.float32(NX * xb) - cx0          # xoff

    opa4 = np.asarray(opacities, np.float32).reshape(P, KC) * np.float32(PI4)
    col = np.asarray(colors, np.float32)
    yu = np.empty((P, 12, KC), np.float32)
    vi = camy * (c * fy) * iss * rz                  # [P, KC]
    fd = ys[None, :, None] * isv[:, None, :] - vi[:, None, :]
    yu[:, 0:NY, :] = np.float32(2.0 / np.sqrt(np.pi)) * np.exp(
        -(fd.astype(np.float32) ** 2))
    for i in range(3):
        yu[:, NY + i, :] = opa4 * col[:, i].reshape(P, KC)
    yu[:, NY + 3, :] = opa4

    return {"yu": yu.astype(np.float16), "xu": xu}


def kernel(positions, colors, opacities, scales, qvec, tvec, intrinsics,
           tile_hw, chunk_gauss, **run_kwargs):
    from concourse.bass_utils import run_bass_kernel_spmd

    tile_hw = int(tile_hw)
    chunk_gauss = int(chunk_gauss)
    assert tile_hw == 8 and positions.shape[0] == N_GAUSS
    n_chunks = -(-N_GAUSS // chunk_gauss)
    eps = np.float32(n_chunks * 1e-8)

    nc = _build()
    in_maps = [
        _core_inputs(c, positions, colors, opacities, scales, qvec, tvec,
                     intrinsics)
        for c in range(N_CORES)
    ]
    res = run_bass_kernel_spmd(nc, in_maps, core_ids=list(range(N_CORES)),
                               **run_kwargs)

    B = np.asarray(qvec).shape[0]
    img = np.zeros((B, 3, NY, 128), np.float32)
    for c in range(N_CORES):
        b, xb = divmod(c, 4)
        o = res.results[c]["out"]               # [4*32 (g,x), 32 (d*8+y)]
        m = o.reshape(4, NX, 32).sum(axis=0)    # [32 x, 32 (d,y)]
        num = m[:, 0:24].T.reshape(3, NY, NX)
        den = m[:, 24:32].T + eps               # [NY, NX]
        img[b, :, :, xb * NX : (xb + 1) * NX] = num / np.maximum(den, 1e-8)
    out = img.reshape(B, 3, NY * 128).reshape(B, 3, 128, 8)
    kernel.last_results = res
    return out
